# revision 1
# baseline (speedup 1.0000x reference)
"""Trainium2 kernel for nn_Group_10: 3x3 replicate-pad conv [4,512,32,32] ->
[4,9728,32,32] (+bias) followed by a per-64-channel-chunk pixel shuffle to
[4,152,256,256].

Sharding: output channels across 8 cores (19 chunks of 64 = 1216 couts each,
padded to 1280 = 10 PE tiles of 128).

Key trick: the pixel shuffle is a pure per-chunk permutation
    out[p, q] = y[cc, h, w],  p = (cc>>1)*8 + (w&7),  q = (cc&1)*128 + 4h + (w>>3)
so the matmul's moving operand visits pixels in order f' = (w&7)*128 + 4h + (w>>3)
(a 3-dim strided AP over the padded image), PSUM comes out already in shuffled
free order, and (with PE output partitions permuted as m = chunkbit*64 +
parity*32 + (cc>>1)) the store to DRAM is a single strided DMA with 512B
contiguous runs.  Matmuls run in float32r (full-rate fp32 PE mode); the BIR
verifier requires every producer feeding an fp32r matmul to write float32r,
so the x/w staging tiles are declared f32r and the DRAM-side APs bitcast.
"""

import numpy as np
from contextlib import ExitStack

import concourse.bass as bass
import concourse.mybir as mybir
import concourse.tile as tile
from concourse import bacc
from concourse.bass_utils import run_bass_kernel_spmd

F32 = mybir.dt.float32
F32R = mybir.dt.float32r

N_CORES = 8
B = 4
CIN = 512
H = W_ = 32
COUT = 9728
NCHUNK = COUT // 64            # 152
CH_PER_CORE = NCHUNK // N_CORES  # 19
COUT_CORE = COUT // N_CORES    # 1216
NTILES = 10                    # 1216 padded to 1280 = 10 tiles of 128
HP = WP = 34                   # replicate-padded image
PIX = HP * WP                  # 1156
NCT = CIN // 128               # 4 cin tiles

# within-tile PE output-partition permutation:
#   partition m = chunkbit*64 + parity*32 + cchalf  <->  cout_in_tile =
#   chunkbit*64 + 2*cchalf + parity
_m = np.arange(128)
_chunkbit, _rem = np.divmod(_m, 64)
_parity, _cchalf = np.divmod(_rem, 32)
COUT_IN_TILE = (_chunkbit * 64 + 2 * _cchalf + _parity).astype(np.int64)  # [128]

_nc_cache = None


def _build_nc(rep=1, skip_mm=False, skip_out=False, order="nbk"):
    """rep>1 wraps the body in an on-device For_i loop — identical I/O
    signature, used by test.py to measure per-iteration HW time by
    differencing wall-clocks against the rep=1 build.  skip_mm/skip_out
    and order are timing-diagnostic ablations (kernel() uses defaults):
    order="wstat" keeps each weight matrix stationary across (n, bk) by
    holding all four batch PSUM tiles live."""
    nc = bacc.Bacc("TRN2", target_bir_lowering=False, debug=False,
                   num_devices=N_CORES)

    xp = nc.dram_tensor("xp", [B, CIN, HP, WP], F32, kind="ExternalInput")
    w = nc.dram_tensor("w", [NTILES, 128, NCT, 9, 128], F32,
                       kind="ExternalInput")
    bias = nc.dram_tensor("bias", [128, NTILES], F32, kind="ExternalInput")
    out = nc.dram_tensor("out", [B, CH_PER_CORE, 256, 256], F32,
                         kind="ExternalOutput")

    with ExitStack() as ctx:
        tc = ctx.enter_context(tile.TileContext(nc))
        xpool = ctx.enter_context(tc.tile_pool(name="xpool", bufs=1))
        wpool = ctx.enter_context(tc.tile_pool(name="wpool", bufs=2))
        opool = ctx.enter_context(tc.tile_pool(name="opool", bufs=3))
        bpool = ctx.enter_context(tc.tile_pool(name="bpool", bufs=1))
        ppool = ctx.enter_context(tc.tile_pool(
            name="ppool", bufs=(4 if order == "wstat" else 3), space="PSUM"))

        def body():
            # x resident in SBUF: partition = cin%128, free = (n, ct) slabs
            # of 1156 pixels in natural (h, w) padded order.
            x_sb = xpool.tile([128, B * NCT * PIX], F32R)
            xrow = x_sb.ap[0][0]
            xt = x_sb.tensor
            xoff0 = x_sb.offset
            for n in range(B):
                for ct in range(NCT):
                    dst = bass.AP(xt, xoff0 + (n * NCT + ct) * PIX,
                                  [[xrow, 128], [1, PIX]])
                    src = bass.AP(xp, (n * CIN + ct * 128) * PIX,
                                  [[PIX, 128], [1, PIX]]).bitcast(F32R)
                    nc.sync.dma_start(dst, src)

            bias_sb = bpool.tile([128, NTILES], F32)
            nc.sync.dma_start(bias_sb, bias[:])

            def rhs_ap(n, ct, tap, bk):
                dy, dx = divmod(tap, 3)
                return bass.AP(
                    xt,
                    xoff0 + (n * NCT + ct) * PIX + dy * WP + dx + 4 * bk,
                    [[xrow, 128], [1, 4], [WP, 32], [8, 4]],
                )

            def store(t, n, psum):
                o_sb = opool.tile([128, 1024], F32)
                nc.vector.tensor_scalar_add(o_sb, psum, bias_sb[:, t:t + 1])
                if skip_out:
                    return
                orow = o_sb.ap[0][0]
                nchunks = 2 if t < NTILES - 1 else 1
                for cb in range(nchunks):
                    src = bass.AP(o_sb.tensor, o_sb.offset + cb * 64 * orow,
                                  [[orow, 64], [128, 8], [1, 128]])
                    base = (n * CH_PER_CORE + 2 * t + cb) * 65536
                    dst = bass.AP(out, base,
                                  [[128, 2], [2048, 32], [256, 8], [1, 128]])
                    nc.sync.dma_start(dst, src)

            w_ap = w[:]
            for t in range(NTILES):
                w_sb = wpool.tile([128, NCT * 9 * 128], F32R)
                nc.sync.dma_start(w_sb, w_ap[t].bitcast(F32R))
                wrow = w_sb.ap[0][0]
                wt = w_sb.tensor
                woff = w_sb.offset

                def lhsT_ap(ct, tap):
                    return bass.AP(wt, woff + (ct * 9 + tap) * 128,
                                   [[wrow, 128], [1, 128]])

                if order == "wstat":
                    psums = [ppool.tile([128, 1024], F32) for _ in range(B)]
                    if not skip_mm:
                        for tap in range(9):
                            for ct in range(NCT):
                                for n in range(B):
                                    for bk in range(2):
                                        nc.tensor.matmul(
                                            psums[n][:, 512 * bk:
                                                     512 * (bk + 1)],
                                            lhsT_ap(ct, tap),
                                            rhs_ap(n, ct, tap, bk),
                                            start=(tap == 0 and ct == 0),
                                            stop=(tap == 8 and ct == 3),
                                        )
                    for n in range(B):
                        store(t, n, psums[n])
                else:
                    for n in range(B):
                        psum = ppool.tile([128, 1024], F32)
                        if not skip_mm:
                            for bk in range(2):  # PSUM bank = r-halves (w&7)
                                for tap in range(9):
                                    for ct in range(NCT):
                                        nc.tensor.matmul(
                                            psum[:, 512 * bk:512 * (bk + 1)],
                                            lhsT_ap(ct, tap),
                                            rhs_ap(n, ct, tap, bk),
                                            start=(tap == 0 and ct == 0),
                                            stop=(tap == 8 and ct == 3),
                                        )
                        store(t, n, psum)

        if rep == 1:
            body()
        else:
            with tc.For_i(0, rep):
                body()

    nc.compile()
    return nc


def _host_prep(x, W, b):
    """Build per-core input maps."""
    xpad = np.pad(np.asarray(x, dtype=np.float32),
                  ((0, 0), (0, 0), (1, 1), (1, 1)), mode="edge")
    xpad = np.ascontiguousarray(xpad)
    W = np.asarray(W, dtype=np.float32)
    b = np.asarray(b, dtype=np.float32)

    in_maps = []
    for i in range(N_CORES):
        Ws = W[i * COUT_CORE:(i + 1) * COUT_CORE]          # [1216,512,3,3]
        Wp = np.zeros((NTILES * 128, CIN, 3, 3), np.float32)
        Wp[:COUT_CORE] = Ws
        gather = (np.arange(NTILES)[:, None] * 128 +
                  COUT_IN_TILE[None, :])                   # [10,128]
        Wg = Wp[gather]                                    # [10,128(m),512,3,3]
        Wg = Wg.reshape(NTILES, 128, NCT, 128, 9)          # [t,m,ct,p,tap]
        w_dev = np.ascontiguousarray(Wg.transpose(0, 3, 2, 4, 1))  # [t,p,ct,tap,m]

        bp = np.zeros((NTILES * 128,), np.float32)
        bp[:COUT_CORE] = b[i * COUT_CORE:(i + 1) * COUT_CORE]
        bias_dev = np.ascontiguousarray(bp[gather].T)      # [128,10]

        in_maps.append({"xp": xpad, "w": w_dev, "bias": bias_dev})
    return in_maps


def _run(in_maps, trace=False):
    global _nc_cache
    if _nc_cache is None:
        _nc_cache = _build_nc()
    return run_bass_kernel_spmd(_nc_cache, in_maps,
                                core_ids=list(range(N_CORES)), trace=trace)


def kernel(x, W, b):
    in_maps = _host_prep(x, W, b)
    res = _run(in_maps)
    outs = [res.results[i]["out"] for i in range(N_CORES)]  # [4,19,256,256]
    full = np.concatenate(outs, axis=1)                     # [4,152,256,256]
    return full



# revision 2
# speedup vs baseline: 1.0684x; 1.0684x over previous
"""Trainium2 kernel for nn_Group_10: 3x3 replicate-pad conv [4,512,32,32] ->
[4,9728,32,32] (+bias) followed by a per-64-channel-chunk pixel shuffle to
[4,152,256,256].

Sharding: output channels across 8 cores (19 chunks of 64 = 1216 couts each,
padded to 1280 = 10 PE tiles of 128).

v6: 1-D Winograd F(2,3) along the row (h / dy) dimension — 1.5x fewer PE
streaming cycles than direct conv.  Per (n, cin-tile) the DVE builds four
B^T row-combination planes R_r[i, c] (i = 16 row-strips of stride 2,
c = 34 padded cols, bf16); the PE accumulates, per Winograd point r,
psum[r][i*32+w] += sum_{dx, ct} (G g)[r, dx]^T . R_r[i, w+dx]  (moving AP
[[34,16],[1,32]] — innermost 32 contiguous).  The A^T output transform
(y[2i] = M0+M1+M2, y[2i+1] = M1-M2-M3), bias add and the pixel-shuffle
permutation all fuse into DVE scalar_tensor_tensor ops writing o_sb in
shuffled order; the store to DRAM stays a strided DMA with 512B runs
(PE output partitions permuted as m = chunkbit*64 + parity*32 + (cc>>1)).
"""

import numpy as np
from contextlib import ExitStack

import concourse.bass as bass
import concourse.mybir as mybir
import concourse.tile as tile
from concourse import bacc
from concourse.bass_utils import run_bass_kernel_spmd

F32 = mybir.dt.float32
BF16 = mybir.dt.bfloat16
ADD = mybir.AluOpType.add
SUB = mybir.AluOpType.subtract

N_CORES = 8
B = 4
CIN = 512
COUT = 9728
NCHUNK = COUT // 64            # 152
CH_PER_CORE = NCHUNK // N_CORES  # 19
COUT_CORE = COUT // N_CORES    # 1216
NTILES = 10                    # 1216 padded to 1280 = 10 tiles of 128
HP = WP = 34                   # replicate-padded image
PIX = HP * WP                  # 1156
NCT = CIN // 128               # 4 cin tiles
RPLANE = 16 * HP               # 544: one R_r plane (16 strips x 34 cols)
XSLAB = 4 * RPLANE             # 2176 per (n, ct)

_m = np.arange(128)
_chunkbit, _rem = np.divmod(_m, 64)
_parity, _cchalf = np.divmod(_rem, 32)
COUT_IN_TILE = (_chunkbit * 64 + 2 * _cchalf + _parity).astype(np.int64)  # [128]

# Winograd F(2,3) matrices
G_WINO = np.array([[1, 0, 0], [.5, .5, .5], [.5, -.5, .5], [0, 0, 1]],
                  dtype=np.float64)

_nc_cache = None


def _build_nc(rep=1, skip_mm=False, skip_out=False):
    nc = bacc.Bacc("TRN2", target_bir_lowering=False, debug=False,
                   num_devices=N_CORES)

    xp = nc.dram_tensor("xp", [B, CIN, HP, WP], BF16, kind="ExternalInput")
    w = nc.dram_tensor("w", [NTILES, 128, NCT, 12, 128], BF16,
                       kind="ExternalInput")
    bias = nc.dram_tensor("bias", [128, NTILES], F32, kind="ExternalInput")
    out = nc.dram_tensor("out", [B, CH_PER_CORE, 256, 256], F32,
                         kind="ExternalOutput")

    with ExitStack() as ctx:
        tc = ctx.enter_context(tile.TileContext(nc))
        spool = ctx.enter_context(tc.tile_pool(name="spool", bufs=3))
        x2pool = ctx.enter_context(tc.tile_pool(name="x2pool", bufs=1))
        wpool = ctx.enter_context(tc.tile_pool(name="wpool", bufs=2))
        opool = ctx.enter_context(tc.tile_pool(name="opool", bufs=3))
        tpool = ctx.enter_context(tc.tile_pool(name="tpool", bufs=3))
        bpool = ctx.enter_context(tc.tile_pool(name="bpool", bufs=1))
        ppool = ctx.enter_context(tc.tile_pool(name="ppool", bufs=2,
                                               space="PSUM"))

        def body():
            # R-plane buffer: per (n, ct) slab of 4 planes x 544, bf16
            x2 = x2pool.tile([128, B * NCT * XSLAB], BF16)
            x2row = x2.ap[0][0]
            x2t = x2.tensor
            x2off = x2.offset

            bias_sb = bpool.tile([128, NTILES], F32)

            def load_transform_slab(n, ct):
                """DMA padded slab, then 4 DVE STT ops build R_0..R_3."""
                slab = spool.tile([128, PIX], BF16)
                srow = slab.ap[0][0]
                st = slab.tensor
                soff = slab.offset
                src = bass.AP(xp, (n * CIN + ct * 128) * PIX,
                              [[PIX, 128], [1, PIX]])
                nc.sync.dma_start(slab, src)

                def rows(r0):   # S[2i + r0], 16 strips x 34 cols
                    return bass.AP(st, soff + r0 * WP,
                                   [[srow, 128], [2 * WP, 16], [1, WP]])

                base = x2off + (n * NCT + ct) * XSLAB
                combos = [(0, 2 * WP, SUB),      # R0 = S[2i]   - S[2i+2]
                          (WP, 2 * WP, ADD),     # R1 = S[2i+1] + S[2i+2]
                          (2 * WP, WP, SUB),     # R2 = S[2i+2] - S[2i+1]
                          (WP, 3 * WP, SUB)]     # R3 = S[2i+1] - S[2i+3]
                for r, (o0, o1, op) in enumerate(combos):
                    dst = bass.AP(x2t, base + r * RPLANE,
                                  [[x2row, 128], [WP, 16], [1, WP]])
                    in0 = bass.AP(st, soff + o0,
                                  [[srow, 128], [2 * WP, 16], [1, WP]])
                    in1 = bass.AP(st, soff + o1,
                                  [[srow, 128], [2 * WP, 16], [1, WP]])
                    nc.vector.scalar_tensor_tensor(dst, in0, 0.0, in1,
                                                   op0=ADD, op1=op)

            def rhs_ap(n, ct, point, dx):
                return bass.AP(
                    x2t,
                    x2off + (n * NCT + ct) * XSLAB + point * RPLANE + dx,
                    [[x2row, 128], [WP, 16], [1, 32]],
                )

            def store(t, n, psum):
                # A^T output transform + bias + pixel-shuffle into o_sb.
                # psum: 4 point planes of 512 (col = strip_i*32 + w).
                o_sb = opool.tile([128, 1024], F32)
                orow = o_sb.ap[0][0]
                prow = psum.ap[0][0]
                pt = psum.tensor
                poff = psum.offset
                def mplane(p0):
                    return bass.AP(pt, poff + 512 * p0,
                                   [[prow, 128], [1, 512]])

                # DVE may read only ONE PSUM operand per op: chain via SBUF.
                for a in range(2):
                    ta = tpool.tile([128, 512], F32)   # copy of M0 / M2
                    tb = tpool.tile([128, 512], F32)   # partial sum
                    trow = tb.ap[0][0]
                    if a == 0:   # y0 = ((M0) + (M1+bias)) + M2
                        nc.vector.tensor_scalar_add(ta, mplane(0), 0.0)
                        nc.vector.scalar_tensor_tensor(
                            tb, mplane(1), bias_sb[:, t:t + 1], ta[:],
                            op0=ADD, op1=ADD)
                        zoff, op2 = 1024, ADD
                    else:        # y1 = ((M1+bias) - (M2)) - M3
                        nc.vector.tensor_scalar_add(ta, mplane(2), 0.0)
                        nc.vector.scalar_tensor_tensor(
                            tb, mplane(1), bias_sb[:, t:t + 1], ta[:],
                            op0=ADD, op1=SUB)
                        zoff, op2 = 1536, SUB
                    # final per bkw: o_sb[f'] = (T + 0) op2 M_z
                    # f' = bkw*512 + i1*128 + 4*(2i+a) + i3,  w = 4bkw+i1+8i3
                    for bkw in range(2):
                        dst = bass.AP(o_sb.tensor,
                                      o_sb.offset + bkw * 512 + 4 * a,
                                      [[orow, 128], [128, 4], [8, 16], [1, 4]])
                        tin = bass.AP(tb.tensor, tb.offset + 4 * bkw,
                                      [[trow, 128], [1, 4], [32, 16], [8, 4]])
                        zin = bass.AP(pt, poff + zoff + 4 * bkw,
                                      [[prow, 128], [1, 4], [32, 16], [8, 4]])
                        nc.vector.scalar_tensor_tensor(
                            dst, tin, 0.0, zin, op0=ADD, op1=op2)
                if skip_out:
                    return
                nchunks = 2 if t < NTILES - 1 else 1
                for cb in range(nchunks):
                    src = bass.AP(o_sb.tensor, o_sb.offset + cb * 64 * orow,
                                  [[orow, 64], [128, 8], [1, 128]])
                    base = (n * CH_PER_CORE + 2 * t + cb) * 65536
                    dst = bass.AP(out, base,
                                  [[128, 2], [2048, 32], [256, 8], [1, 128]])
                    nc.sync.dma_start(dst, src)

            w_ap = w[:]
            for t in range(NTILES):
                w_sb = wpool.tile([128, NCT * 12 * 128], BF16)
                wrow = w_sb.ap[0][0]
                wt = w_sb.tensor
                woff = w_sb.offset
                for ct in range(NCT):
                    dst = bass.AP(wt, woff + ct * 12 * 128,
                                  [[wrow, 128], [1, 12 * 128]])
                    nc.sync.dma_start(dst, w_ap[t][:, ct])
                if t == 0:
                    # first compute group's inputs, then bias, then the rest
                    for n in range(B):
                        for ct in range(NCT):
                            load_transform_slab(n, ct)
                    nc.sync.dma_start(bias_sb, bias[:])

                def lhsT_ap(ct, point, dx):
                    return bass.AP(wt,
                                   woff + (ct * 12 + point * 3 + dx) * 128,
                                   [[wrow, 128], [1, 128]])

                for n in range(B):
                    psum = ppool.tile([128, 2048], F32)
                    if not skip_mm:
                        for point in range(4):
                            for dx in range(3):
                                for ct in range(NCT):
                                    nc.tensor.matmul(
                                        psum[:, 512 * point:512 * (point + 1)],
                                        lhsT_ap(ct, point, dx),
                                        rhs_ap(n, ct, point, dx),
                                        start=(dx == 0 and ct == 0),
                                        stop=(dx == 2 and ct == 3),
                                    )
                    store(t, n, psum)

        if rep == 1:
            body()
        else:
            with tc.For_i(0, rep):
                body()

    nc.compile()
    return nc


def _host_prep(x, W, b):
    """Build per-core input maps."""
    import ml_dtypes
    xpad = np.pad(np.asarray(x, dtype=np.float32),
                  ((0, 0), (0, 0), (1, 1), (1, 1)), mode="edge")
    xpad = np.ascontiguousarray(xpad.astype(ml_dtypes.bfloat16))
    W = np.asarray(W, dtype=np.float32)
    b = np.asarray(b, dtype=np.float32)

    in_maps = []
    for i in range(N_CORES):
        Ws = W[i * COUT_CORE:(i + 1) * COUT_CORE]          # [1216,512,3,3]
        Wp = np.zeros((NTILES * 128, CIN, 3, 3), np.float32)
        Wp[:COUT_CORE] = Ws
        gather = (np.arange(NTILES)[:, None] * 128 +
                  COUT_IN_TILE[None, :])                   # [10,128]
        Wg = Wp[gather]                                    # [10,128(m),512,3,3]
        # 1-D Winograd weight transform along dy: U = G @ g  -> [4, 3]
        U = np.einsum('ad,tmcdx->tmcax',
                      G_WINO, Wg.reshape(NTILES, 128, CIN, 3, 3))
        U = U.reshape(NTILES, 128, NCT, 128, 4, 3)         # [t,m,ct,p,a,dx]
        w_dev = np.ascontiguousarray(
            U.transpose(0, 3, 2, 4, 5, 1)                  # [t,p,ct,a,dx,m]
            .reshape(NTILES, 128, NCT, 12, 128)
            .astype(ml_dtypes.bfloat16))

        bp = np.zeros((NTILES * 128,), np.float32)
        bp[:COUT_CORE] = b[i * COUT_CORE:(i + 1) * COUT_CORE]
        bias_dev = np.ascontiguousarray(bp[gather].T)      # [128,10]

        in_maps.append({"xp": xpad, "w": w_dev, "bias": bias_dev})
    return in_maps


def _run(in_maps, trace=False):
    global _nc_cache
    if _nc_cache is None:
        _nc_cache = _build_nc()
    return run_bass_kernel_spmd(_nc_cache, in_maps,
                                core_ids=list(range(N_CORES)), trace=trace)


def kernel(x, W, b):
    in_maps = _host_prep(x, W, b)
    res = _run(in_maps)
    outs = [res.results[i]["out"] for i in range(N_CORES)]  # [4,19,256,256]
    full = np.concatenate(outs, axis=1)                     # [4,152,256,256]
    return full
